# revision 11
# baseline (speedup 1.0000x reference)
"""Trainium2 Bass kernel for nn_Decoder (ragged sinusoidal-query decoder).

Math (per sample b):
    n      = z[b, 64]                       count in [1, 16]
    zf     = z[b, :64] viewed as [8, 8]
    query  = pos_enc(16, 128) @ Wq_w.T      [16, 8]
    x[b]   = (query @ zf @ map_w.T + map_b) * mask    [16, 128]
    mask   = arange(16) < n

The whole thing collapses to ONE matmul per sample:
    x[b].flatten() = z_aug[b] @ W_aug  masked per (b, p)
where W_aug[65, 2048] combines query and map_w (plus a bias row hit by a
ones-column in z_aug), and mask_flat[b, j] = (j < 128 * n[b]).

For near-fp32 accuracy at bf16 PE speed the product is computed as
    z_hi@W_hi + z_lo@W_hi + z_hi@W_lo      (hi/lo bf16 split, host-prepped)
packed into TWO K=97 matmuls by row concatenation:
    A: [z_hi(64); 1; z_lo[0:32]]  @  [W_hi; bias_hi; W_hi[0:32]]
    B: [z_lo[32:64]; 1; z_hi(64)] @  [W_hi[32:64]; bias_lo; W_lo]
(K <= 64 runs the PE at half clock on trn2; K=97 is full rate.)

Device kernel per core (8192 rows, data-parallel over 8 cores):
  - host packs A/B lhsT tiles interleaved: one [97, 256] DMA per 128-row
    tile on the ACT HWDGE ring (separate from the output ring).
  - TensorE: 8 bf16 matmuls per tile (A+B accumulating per 512-col bank).
  - ScalarE: mini-mask [128, 16] = Sigmoid(-64*p + (64n - 32)) -> 0/1.
  - VectorE: one tensor_mul per tile evicts PSUM [128, 2048] -> SBUF,
    reading the mini-mask through a stride-0 broadcast AP.
  - DMA out tile [128, 2048] fp32 on the sync HWDGE ring.
"""

import numpy as np

B = 65536
DIM = 128
WD = 8
P = 16
N_CORES = 8
ROWS = B // N_CORES          # 8192 rows per core
RTILE = 128                  # rows per on-chip tile
NTILES = ROWS // RTILE       # 64
KP = 97                      # packed contraction size (both passes)
NOUT = P * DIM               # 2048 output columns

_CACHE = {}


def _build_nc():
    import concourse.bass as bass
    import concourse.mybir as mybir
    import concourse.tile as tile
    from concourse import bacc
    from contextlib import ExitStack

    f32 = mybir.dt.float32
    bf16 = mybir.dt.bfloat16
    nc = bacc.Bacc(None, target_bir_lowering=False)

    za = nc.declare_dram_parameter("za", [KP, ROWS], bf16, isOutput=False)
    zb = nc.declare_dram_parameter("zb", [KP, ROWS], bf16, isOutput=False)
    wa = nc.declare_dram_parameter("wa", [KP, NOUT], bf16, isOutput=False)
    wb = nc.declare_dram_parameter("wb", [KP, NOUT], bf16, isOutput=False)
    nsc = nc.declare_dram_parameter("nsc", [RTILE, NTILES], f32, isOutput=False)
    out = nc.declare_dram_parameter("out", [ROWS, NOUT], f32, isOutput=True)

    with tile.TileContext(nc) as tc, ExitStack() as ctx:
        singles = ctx.enter_context(tc.tile_pool(name="singles", bufs=1))
        zpool = ctx.enter_context(tc.tile_pool(name="zpool", bufs=12))
        mmps = ctx.enter_context(tc.tile_pool(name="mmps", bufs=2, space="PSUM"))
        maskp = ctx.enter_context(tc.tile_pool(name="maskp", bufs=4))
        outp = ctx.enter_context(tc.tile_pool(name="outp", bufs=4))

        wa_sb = singles.tile([KP, NOUT], bf16)
        nc.sync.dma_start(out=wa_sb, in_=wa[:, :])
        wb_sb = singles.tile([KP, NOUT], bf16)
        nc.sync.dma_start(out=wb_sb, in_=wb[:, :])
        nsc_sb = singles.tile([RTILE, NTILES], f32)
        nc.sync.dma_start(out=nsc_sb, in_=nsc[:, :])
        iota16 = singles.tile([128, P], f32)
        nc.gpsimd.iota(iota16[:, :], [[1, P]], channel_multiplier=0,
                       allow_small_or_imprecise_dtypes=True)

        for i in range(NTILES):
            r0 = i * RTILE
            za_t = zpool.tile([KP, RTILE], bf16, tag="za")
            nc.scalar.dma_start(out=za_t, in_=za[:, r0 : r0 + RTILE])
            zb_t = zpool.tile([KP, RTILE], bf16, tag="zb")
            nc.scalar.dma_start(out=zb_t, in_=zb[:, r0 : r0 + RTILE])

            mask_t = maskp.tile([RTILE, P], f32)
            nc.scalar.activation(
                out=mask_t,
                in_=iota16,
                func=mybir.ActivationFunctionType.Sigmoid,
                bias=nsc_sb[:, i : i + 1],
                scale=-64.0,
            )
            mask_bcast = bass.AP(
                tensor=mask_t.tensor,
                offset=mask_t.offset,
                ap=[mask_t.ap[0], [1, P], [0, DIM]],
            )

            mm_ps = mmps.tile([RTILE, NOUT], f32)
            for h in range(4):
                w0 = h * 512
                pslice = mm_ps[:, w0 : w0 + 512]
                nc.tensor.matmul(
                    out=pslice, lhsT=za_t, rhs=wa_sb[:, w0 : w0 + 512],
                    start=True, stop=False,
                )
                nc.tensor.matmul(
                    out=pslice, lhsT=zb_t, rhs=wb_sb[:, w0 : w0 + 512],
                    start=False, stop=True,
                )

            x_t = outp.tile([RTILE, NOUT], f32)
            nc.vector.tensor_mul(
                x_t.rearrange("r (p o) -> r p o", p=P),
                mm_ps.rearrange("r (p o) -> r p o", p=P),
                mask_bcast,
            )

            nc.sync.dma_start(out=out[i * RTILE : (i + 1) * RTILE, :], in_=x_t)

    nc.finalize()
    return nc


def _sinusoid_pos_enc(length, dim):
    pos = np.arange(length, dtype=np.float32)[:, None]
    i = np.arange(0, dim, 2, dtype=np.float32)
    inv_freq = np.exp(-np.log(10000.0) * i / dim).astype(np.float32)
    ang = pos * inv_freq
    pe = np.zeros((length, dim), dtype=np.float32)
    pe[:, 0::2] = np.sin(ang)
    pe[:, 1::2] = np.cos(ang)
    return pe


def _host_prep(z, Wq_w, map_w, map_b):
    import ml_dtypes

    bf16 = ml_dtypes.bfloat16
    n = z[:, 64].astype(np.int32)                       # [B], 1..16

    zf_t = np.ascontiguousarray(z[:, :64].T)            # [64, B] fp32
    zf_hi = zf_t.astype(bf16)
    zf_lo = (zf_t - zf_hi.astype(np.float32)).astype(bf16)

    # pass A lhsT rows: [z_hi(64); ones; z_lo[0:32]]   (97)
    za = np.empty((KP, B), dtype=bf16)
    za[:64] = zf_hi
    za[64] = np.float32(1.0)
    za[65:] = zf_lo[0:32]
    # pass B lhsT rows: [z_lo[32:64]; ones; z_hi(64)]  (97)
    zb = np.empty((KP, B), dtype=bf16)
    zb[0:32] = zf_lo[32:64]
    zb[32] = np.float32(1.0)
    zb[33:] = zf_hi

    # W_comb: [64, 2048]; row (k*8+d), col (p*128+o) = query[p,k]*map_w[o,d]
    query = _sinusoid_pos_enc(P, DIM) @ Wq_w.T          # [16, 8]
    w_comb = np.einsum("pk,od->kdpo", query.astype(np.float64),
                       map_w.astype(np.float64)).reshape(64, NOUT).astype(np.float32)
    w_hi = w_comb.astype(bf16)
    w_lo = (w_comb - w_hi.astype(np.float32)).astype(bf16)
    bias = np.tile(map_b, P).astype(np.float32)         # [2048]
    bias_hi = bias.astype(bf16)
    bias_lo = (bias - bias_hi.astype(np.float32)).astype(bf16)

    wa = np.empty((KP, NOUT), dtype=bf16)
    wa[:64] = w_hi
    wa[64] = bias_hi
    wa[65:] = w_hi[0:32]
    wb = np.empty((KP, NOUT), dtype=bf16)
    wb[0:32] = w_hi[32:64]
    wb[32] = bias_lo
    wb[33:] = w_lo

    nscaled = (64.0 * n - 32.0).astype(np.float32)

    in_maps = []
    for c in range(N_CORES):
        s = slice(c * ROWS, (c + 1) * ROWS)
        in_maps.append({
            "za": np.ascontiguousarray(za[:, s]),
            "zb": np.ascontiguousarray(zb[:, s]),
            "wa": wa,
            "wb": wb,
            "nsc": np.ascontiguousarray(
                nscaled[s].reshape(NTILES, RTILE).T),
        })
    return in_maps, n


def kernel(z, Wq_w, map_w, map_b):
    from concourse.bass_utils import run_bass_kernel_spmd

    z = np.asarray(z, dtype=np.float32)
    Wq_w = np.asarray(Wq_w, dtype=np.float32)
    map_w = np.asarray(map_w, dtype=np.float32)
    map_b = np.asarray(map_b, dtype=np.float32)

    in_maps, n = _host_prep(z, Wq_w, map_w, map_b)

    if "nc" not in _CACHE:
        _CACHE["nc"] = _build_nc()
    nc = _CACHE["nc"]

    res = run_bass_kernel_spmd(nc, in_maps, core_ids=list(range(N_CORES)))
    x = np.concatenate([r["out"] for r in res.results], axis=0).reshape(B, P, DIM)

    mask = np.arange(P, dtype=np.int32)[None, :] < n[:, None]
    batch = np.ascontiguousarray(
        np.broadcast_to(np.arange(B, dtype=np.int32)[:, None], (B, P))
    )
    return x, mask, batch


# revision 12
# speedup vs baseline: 1.5686x; 1.5686x over previous
"""Trainium2 Bass kernel for nn_Decoder (ragged sinusoidal-query decoder).

Math (per sample b):
    n      = z[b, 64]                       count in [1, 16]
    zf     = z[b, :64] viewed as [8, 8]
    query  = pos_enc(16, 128) @ Wq_w.T      [16, 8]
    x[b]   = (query @ zf @ map_w.T + map_b) * mask    [16, 128]
    mask   = arange(16) < n

The whole thing collapses to ONE matmul per sample:
    x[b].flatten() = z_aug[b] @ W_aug  masked per (b, p)
where W_aug[65, 2048] combines query and map_w (plus a bias row hit by a
ones-column in z_aug), and mask_flat[b, j] = (j < 128 * n[b]).

For near-fp32 accuracy at bf16 PE speed the product is computed as
    z_hi@W_hi + z_lo@W_hi + z_hi@W_lo      (hi/lo bf16 split, host-prepped)
packed into TWO K=97 matmuls by row concatenation:
    A: [z_hi(64); 1; z_lo[0:32]]  @  [W_hi; bias_hi; W_hi[0:32]]
    B: [z_lo[32:64]; 1; z_hi(64)] @  [W_hi[32:64]; bias_lo; W_lo]
(K <= 64 runs the PE at half clock on trn2; K=97 is full rate.)

Device kernel per core (8192 rows, data-parallel over 8 cores):
  - host packs A/B lhsT tiles interleaved: one [97, 256] DMA per 128-row
    tile on the ACT HWDGE ring (separate from the output ring).
  - TensorE: 8 bf16 matmuls per tile (A+B accumulating per 512-col bank).
  - ScalarE: mini-mask [128, 16] = Sigmoid(-64*p + (64n - 32)) -> 0/1.
  - VectorE: one tensor_mul per tile evicts PSUM [128, 2048] -> SBUF,
    reading the mini-mask through a stride-0 broadcast AP.
  - DMA out tile [128, 2048] fp32 on the sync HWDGE ring.
"""

import numpy as np

B = 65536
DIM = 128
WD = 8
P = 16
N_CORES = 8
ROWS = B // N_CORES          # 8192 rows per core
RTILE = 128                  # rows per on-chip tile
NTILES = ROWS // RTILE       # 64
KA = 96                      # pass A contraction (even partition counts
KB = 98                      # pass B contraction  spread DMA rings; odd pin to one)
NOUT = P * DIM               # 2048 output columns

_CACHE = {}


def _build_nc():
    import concourse.bass as bass
    import concourse.mybir as mybir
    import concourse.tile as tile
    from concourse import bacc
    from contextlib import ExitStack

    f32 = mybir.dt.float32
    bf16 = mybir.dt.bfloat16
    nc = bacc.Bacc(None, target_bir_lowering=False)

    za = nc.declare_dram_parameter("za", [KA, ROWS], bf16, isOutput=False)
    zb = nc.declare_dram_parameter("zb", [KB, ROWS], bf16, isOutput=False)
    wa = nc.declare_dram_parameter("wa", [KA, NOUT], bf16, isOutput=False)
    wb = nc.declare_dram_parameter("wb", [KB, NOUT], bf16, isOutput=False)
    nsc = nc.declare_dram_parameter("nsc", [RTILE, NTILES], f32, isOutput=False)
    out = nc.declare_dram_parameter("out", [ROWS, NOUT], f32, isOutput=True)

    with tile.TileContext(nc) as tc, ExitStack() as ctx:
        singles = ctx.enter_context(tc.tile_pool(name="singles", bufs=1))
        zpool = ctx.enter_context(tc.tile_pool(name="zpool", bufs=12))
        mmps = ctx.enter_context(tc.tile_pool(name="mmps", bufs=2, space="PSUM"))
        maskp = ctx.enter_context(tc.tile_pool(name="maskp", bufs=4))
        outp = ctx.enter_context(tc.tile_pool(name="outp", bufs=4))

        wa_sb = singles.tile([KA, NOUT], bf16)
        nc.sync.dma_start(out=wa_sb, in_=wa[:, :])
        wb_sb = singles.tile([KB, NOUT], bf16)
        nc.sync.dma_start(out=wb_sb, in_=wb[:, :])
        nsc_sb = singles.tile([RTILE, NTILES], f32)
        nc.sync.dma_start(out=nsc_sb, in_=nsc[:, :])
        iota16 = singles.tile([128, P], f32)
        nc.gpsimd.iota(iota16[:, :], [[1, P]], channel_multiplier=0,
                       allow_small_or_imprecise_dtypes=True)

        for i in range(NTILES):
            r0 = i * RTILE
            za_t = zpool.tile([KA, RTILE], bf16, tag="za")
            nc.scalar.dma_start(out=za_t, in_=za[:, r0 : r0 + RTILE])
            zb_t = zpool.tile([KB, RTILE], bf16, tag="zb")
            nc.scalar.dma_start(out=zb_t, in_=zb[:, r0 : r0 + RTILE])

            mask_t = maskp.tile([RTILE, P], f32)
            nc.scalar.activation(
                out=mask_t,
                in_=iota16,
                func=mybir.ActivationFunctionType.Sigmoid,
                bias=nsc_sb[:, i : i + 1],
                scale=-64.0,
            )
            mask_bcast = bass.AP(
                tensor=mask_t.tensor,
                offset=mask_t.offset,
                ap=[mask_t.ap[0], [1, P], [0, DIM]],
            )

            mm_ps = mmps.tile([RTILE, NOUT], f32)
            for h in range(4):
                w0 = h * 512
                pslice = mm_ps[:, w0 : w0 + 512]
                nc.tensor.matmul(
                    out=pslice, lhsT=za_t, rhs=wa_sb[:, w0 : w0 + 512],
                    start=True, stop=False,
                )
                nc.tensor.matmul(
                    out=pslice, lhsT=zb_t, rhs=wb_sb[:, w0 : w0 + 512],
                    start=False, stop=True,
                )

            x_t = outp.tile([RTILE, NOUT], f32)
            nc.vector.tensor_mul(
                x_t.rearrange("r (p o) -> r p o", p=P),
                mm_ps.rearrange("r (p o) -> r p o", p=P),
                mask_bcast,
            )

            nc.sync.dma_start(out=out[i * RTILE : (i + 1) * RTILE, :], in_=x_t)

    nc.finalize()
    return nc


def _sinusoid_pos_enc(length, dim):
    pos = np.arange(length, dtype=np.float32)[:, None]
    i = np.arange(0, dim, 2, dtype=np.float32)
    inv_freq = np.exp(-np.log(10000.0) * i / dim).astype(np.float32)
    ang = pos * inv_freq
    pe = np.zeros((length, dim), dtype=np.float32)
    pe[:, 0::2] = np.sin(ang)
    pe[:, 1::2] = np.cos(ang)
    return pe


def _host_prep(z, Wq_w, map_w, map_b):
    import ml_dtypes

    bf16 = ml_dtypes.bfloat16
    n = z[:, 64].astype(np.int32)                       # [B], 1..16

    zf_t = np.ascontiguousarray(z[:, :64].T)            # [64, B] fp32
    zf_hi = zf_t.astype(bf16)
    zf_lo = (zf_t - zf_hi.astype(np.float32)).astype(bf16)

    # pass A lhsT rows: [z_hi(64); ones; z_lo[0:31]]   (96)
    za = np.empty((KA, B), dtype=bf16)
    za[:64] = zf_hi
    za[64] = np.float32(1.0)
    za[65:] = zf_lo[0:31]
    # pass B lhsT rows: [z_lo[31:64]; ones; z_hi(64)]  (98)
    zb = np.empty((KB, B), dtype=bf16)
    zb[0:33] = zf_lo[31:64]
    zb[33] = np.float32(1.0)
    zb[34:] = zf_hi

    # W_comb: [64, 2048]; row (k*8+d), col (p*128+o) = query[p,k]*map_w[o,d]
    query = _sinusoid_pos_enc(P, DIM) @ Wq_w.T          # [16, 8]
    w_comb = np.einsum("pk,od->kdpo", query.astype(np.float64),
                       map_w.astype(np.float64)).reshape(64, NOUT).astype(np.float32)
    w_hi = w_comb.astype(bf16)
    w_lo = (w_comb - w_hi.astype(np.float32)).astype(bf16)
    bias = np.tile(map_b, P).astype(np.float32)         # [2048]
    bias_hi = bias.astype(bf16)
    bias_lo = (bias - bias_hi.astype(np.float32)).astype(bf16)

    wa = np.empty((KA, NOUT), dtype=bf16)
    wa[:64] = w_hi
    wa[64] = bias_hi
    wa[65:] = w_hi[0:31]
    wb = np.empty((KB, NOUT), dtype=bf16)
    wb[0:33] = w_hi[31:64]
    wb[33] = bias_lo
    wb[34:] = w_lo

    nscaled = (64.0 * n - 32.0).astype(np.float32)

    in_maps = []
    for c in range(N_CORES):
        s = slice(c * ROWS, (c + 1) * ROWS)
        in_maps.append({
            "za": np.ascontiguousarray(za[:, s]),
            "zb": np.ascontiguousarray(zb[:, s]),
            "wa": wa,
            "wb": wb,
            "nsc": np.ascontiguousarray(
                nscaled[s].reshape(NTILES, RTILE).T),
        })
    return in_maps, n


def kernel(z, Wq_w, map_w, map_b):
    from concourse.bass_utils import run_bass_kernel_spmd

    z = np.asarray(z, dtype=np.float32)
    Wq_w = np.asarray(Wq_w, dtype=np.float32)
    map_w = np.asarray(map_w, dtype=np.float32)
    map_b = np.asarray(map_b, dtype=np.float32)

    in_maps, n = _host_prep(z, Wq_w, map_w, map_b)

    if "nc" not in _CACHE:
        _CACHE["nc"] = _build_nc()
    nc = _CACHE["nc"]

    res = run_bass_kernel_spmd(nc, in_maps, core_ids=list(range(N_CORES)))
    x = np.concatenate([r["out"] for r in res.results], axis=0).reshape(B, P, DIM)

    mask = np.arange(P, dtype=np.int32)[None, :] < n[:, None]
    batch = np.ascontiguousarray(
        np.broadcast_to(np.arange(B, dtype=np.int32)[:, None], (B, P))
    )
    return x, mask, batch


# revision 13
# speedup vs baseline: 1.7884x; 1.1401x over previous
"""Trainium2 Bass kernel for nn_Decoder (ragged sinusoidal-query decoder).

Math (per sample b):
    n      = z[b, 64]                       count in [1, 16]
    zf     = z[b, :64] viewed as [8, 8]
    query  = pos_enc(16, 128) @ Wq_w.T      [16, 8]
    x[b]   = (query @ zf @ map_w.T + map_b) * mask    [16, 128]
    mask   = arange(16) < n

The whole thing collapses to ONE matmul per sample:
    x[b].flatten() = z_aug[b] @ W_aug  masked per (b, p)
where W_aug[65, 2048] combines query and map_w (plus a bias row hit by a
ones-column in z_aug), and mask_flat[b, j] = (j < 128 * n[b]).

For near-fp32 accuracy at bf16 PE speed the product is computed as
    z_hi@W_hi + z_lo@W_hi + z_hi@W_lo      (hi/lo bf16 split, host-prepped)
packed into TWO K=97 matmuls by row concatenation:
    A: [z_hi(64); 1; z_lo[0:32]]  @  [W_hi; bias_hi; W_hi[0:32]]
    B: [z_lo[32:64]; 1; z_hi(64)] @  [W_hi[32:64]; bias_lo; W_lo]
(K <= 64 runs the PE at half clock on trn2; K=97 is full rate.)

Device kernel per core (8192 rows, data-parallel over 8 cores):
  - host packs A/B lhsT tiles interleaved: one [97, 256] DMA per 128-row
    tile on the ACT HWDGE ring (separate from the output ring).
  - TensorE: 8 bf16 matmuls per tile (A+B accumulating per 512-col bank).
  - ScalarE: mini-mask [128, 16] = Sigmoid(-64*p + (64n - 32)) -> 0/1.
  - VectorE: one tensor_mul per tile evicts PSUM [128, 2048] -> SBUF,
    reading the mini-mask through a stride-0 broadcast AP.
  - DMA out tile [128, 2048] fp32 on the sync HWDGE ring.
"""

import numpy as np

B = 65536
DIM = 128
WD = 8
P = 16
N_CORES = 8
ROWS = B // N_CORES          # 8192 rows per core
RTILE = 128                  # rows per on-chip tile
NTILES = ROWS // RTILE       # 64
KA = 96                      # pass A contraction (even partition counts
KB = 98                      # pass B contraction  spread DMA rings; odd pin to one)
NOUT = P * DIM               # 2048 output columns

_CACHE = {}


def _build_nc():
    import concourse.bass as bass
    import concourse.mybir as mybir
    import concourse.tile as tile
    from concourse import bacc
    from contextlib import ExitStack

    f32 = mybir.dt.float32
    bf16 = mybir.dt.bfloat16
    nc = bacc.Bacc(None, target_bir_lowering=False)

    za = nc.declare_dram_parameter("za", [KA, ROWS], bf16, isOutput=False)
    zb = nc.declare_dram_parameter("zb", [KB, ROWS], bf16, isOutput=False)
    wa = nc.declare_dram_parameter("wa", [KA, NOUT], bf16, isOutput=False)
    wb = nc.declare_dram_parameter("wb", [KB, NOUT], bf16, isOutput=False)
    nsc = nc.declare_dram_parameter("nsc", [RTILE, NTILES], f32, isOutput=False)
    out = nc.declare_dram_parameter("out", [ROWS, NOUT], f32, isOutput=True)

    with tile.TileContext(nc) as tc, ExitStack() as ctx:
        singles = ctx.enter_context(tc.tile_pool(name="singles", bufs=1))
        zpool = ctx.enter_context(tc.tile_pool(name="zpool", bufs=16))
        mmps = ctx.enter_context(tc.tile_pool(name="mmps", bufs=2, space="PSUM"))
        maskp = ctx.enter_context(tc.tile_pool(name="maskp", bufs=4))
        outp = ctx.enter_context(tc.tile_pool(name="outp", bufs=4))

        wa_sb = singles.tile([KA, NOUT], bf16)
        nc.sync.dma_start(out=wa_sb, in_=wa[:, :])
        wb_sb = singles.tile([KB, NOUT], bf16)
        nc.sync.dma_start(out=wb_sb, in_=wb[:, :])
        nsc_sb = singles.tile([RTILE, NTILES], f32)
        nc.sync.dma_start(out=nsc_sb, in_=nsc[:, :])
        iota16 = singles.tile([128, P], f32)
        nc.gpsimd.iota(iota16[:, :], [[1, P]], channel_multiplier=0,
                       allow_small_or_imprecise_dtypes=True)

        for i in range(NTILES):
            r0 = i * RTILE
            za_t = zpool.tile([KA, RTILE], bf16, tag="za")
            nc.scalar.dma_start(out=za_t, in_=za[:, r0 : r0 + RTILE])
            zb_t = zpool.tile([KB, RTILE], bf16, tag="zb")
            nc.scalar.dma_start(out=zb_t, in_=zb[:, r0 : r0 + RTILE])

            mask_t = maskp.tile([RTILE, P], f32)
            nc.scalar.activation(
                out=mask_t,
                in_=iota16,
                func=mybir.ActivationFunctionType.Sigmoid,
                bias=nsc_sb[:, i : i + 1],
                scale=-64.0,
            )


            mm_ps = mmps.tile([RTILE, NOUT], f32)
            for h in range(4):
                w0 = h * 512
                pslice = mm_ps[:, w0 : w0 + 512]
                nc.tensor.matmul(
                    out=pslice, lhsT=za_t, rhs=wa_sb[:, w0 : w0 + 512],
                    start=True, stop=False,
                )
                nc.tensor.matmul(
                    out=pslice, lhsT=zb_t, rhs=wb_sb[:, w0 : w0 + 512],
                    start=False, stop=True,
                )

            x_t = outp.tile([RTILE, NOUT], f32)
            for u in range(2):
                mu = mask_t[:, u * (P // 2) : (u + 1) * (P // 2)]
                mask_bcast = bass.AP(
                    tensor=mu.tensor,
                    offset=mu.offset,
                    ap=[mu.ap[0], [1, P // 2], [0, DIM]],
                )
                half = slice(u * (NOUT // 2), (u + 1) * (NOUT // 2))
                nc.vector.tensor_mul(
                    x_t[:, half].rearrange("r (p o) -> r p o", p=P // 2),
                    mm_ps[:, half].rearrange("r (p o) -> r p o", p=P // 2),
                    mask_bcast,
                )

            nc.sync.dma_start(out=out[i * RTILE : (i + 1) * RTILE, :], in_=x_t)

    nc.finalize()
    return nc


def _sinusoid_pos_enc(length, dim):
    pos = np.arange(length, dtype=np.float32)[:, None]
    i = np.arange(0, dim, 2, dtype=np.float32)
    inv_freq = np.exp(-np.log(10000.0) * i / dim).astype(np.float32)
    ang = pos * inv_freq
    pe = np.zeros((length, dim), dtype=np.float32)
    pe[:, 0::2] = np.sin(ang)
    pe[:, 1::2] = np.cos(ang)
    return pe


def _host_prep(z, Wq_w, map_w, map_b):
    import ml_dtypes

    bf16 = ml_dtypes.bfloat16
    n = z[:, 64].astype(np.int32)                       # [B], 1..16

    zf_t = np.ascontiguousarray(z[:, :64].T)            # [64, B] fp32
    zf_hi = zf_t.astype(bf16)
    zf_lo = (zf_t - zf_hi.astype(np.float32)).astype(bf16)

    # pass A lhsT rows: [z_hi(64); ones; z_lo[0:31]]   (96)
    za = np.empty((KA, B), dtype=bf16)
    za[:64] = zf_hi
    za[64] = np.float32(1.0)
    za[65:] = zf_lo[0:31]
    # pass B lhsT rows: [z_lo[31:64]; ones; z_hi(64)]  (98)
    zb = np.empty((KB, B), dtype=bf16)
    zb[0:33] = zf_lo[31:64]
    zb[33] = np.float32(1.0)
    zb[34:] = zf_hi

    # W_comb: [64, 2048]; row (k*8+d), col (p*128+o) = query[p,k]*map_w[o,d]
    query = _sinusoid_pos_enc(P, DIM) @ Wq_w.T          # [16, 8]
    w_comb = np.einsum("pk,od->kdpo", query.astype(np.float64),
                       map_w.astype(np.float64)).reshape(64, NOUT).astype(np.float32)
    w_hi = w_comb.astype(bf16)
    w_lo = (w_comb - w_hi.astype(np.float32)).astype(bf16)
    bias = np.tile(map_b, P).astype(np.float32)         # [2048]
    bias_hi = bias.astype(bf16)
    bias_lo = (bias - bias_hi.astype(np.float32)).astype(bf16)

    wa = np.empty((KA, NOUT), dtype=bf16)
    wa[:64] = w_hi
    wa[64] = bias_hi
    wa[65:] = w_hi[0:31]
    wb = np.empty((KB, NOUT), dtype=bf16)
    wb[0:33] = w_hi[31:64]
    wb[33] = bias_lo
    wb[34:] = w_lo

    nscaled = (64.0 * n - 32.0).astype(np.float32)

    in_maps = []
    for c in range(N_CORES):
        s = slice(c * ROWS, (c + 1) * ROWS)
        in_maps.append({
            "za": np.ascontiguousarray(za[:, s]),
            "zb": np.ascontiguousarray(zb[:, s]),
            "wa": wa,
            "wb": wb,
            "nsc": np.ascontiguousarray(
                nscaled[s].reshape(NTILES, RTILE).T),
        })
    return in_maps, n


def kernel(z, Wq_w, map_w, map_b):
    from concourse.bass_utils import run_bass_kernel_spmd

    z = np.asarray(z, dtype=np.float32)
    Wq_w = np.asarray(Wq_w, dtype=np.float32)
    map_w = np.asarray(map_w, dtype=np.float32)
    map_b = np.asarray(map_b, dtype=np.float32)

    in_maps, n = _host_prep(z, Wq_w, map_w, map_b)

    if "nc" not in _CACHE:
        _CACHE["nc"] = _build_nc()
    nc = _CACHE["nc"]

    res = run_bass_kernel_spmd(nc, in_maps, core_ids=list(range(N_CORES)))
    x = np.concatenate([r["out"] for r in res.results], axis=0).reshape(B, P, DIM)

    mask = np.arange(P, dtype=np.int32)[None, :] < n[:, None]
    batch = np.ascontiguousarray(
        np.broadcast_to(np.arange(B, dtype=np.int32)[:, None], (B, P))
    )
    return x, mask, batch
